# revision 8
# baseline (speedup 1.0000x reference)
"""Trainium2 Bass kernel for CrossAttention (b=4, p=8, n=512, dim=512, 8 heads x 64).

Sharding: the 32 independent (b, p) slices are split 4-per-core across 8
NeuronCores (pure data parallel, no collectives). Weights are replicated.

Host-side prep (inside kernel()): inputs are transposed per-slice to
[dim, n] and cast to bf16, so the device never transposes activations;
weights are cast to bf16 on the host too.

Per-slice device dataflow (all SBUF tiles are [partition, free]):
  - qT = Wq-blocks^T @ xqT, kT likewise; v = xkvT-blocks^T @ Wv  (PE)
  - per head: ST[j, i] = kT_h-block^T @ qT_h -> exp (ACT, scale=1/8) -> PT bf16
    (no max subtraction: scores are ~N(0,1), exp cannot overflow)
  - PV for a head pair is issued column-group-paired so PE overlaps:
    even head outT -> PSUM[0:64] while odd head's l-matmul (ones[128,33])
    lands in PSUM[64:97] of a second bank, and vice versa. l rows at the
    two quadrant bases let a DVE stream_shuffle broadcast l across all 64
    partitions of the head's parity range, all partition-aligned.
  - normalize: outT_h *= 1/l_h (DVE reciprocal + stream_shuffle + mul)
  - final: fin[i, f] = outT-blocks^T @ Wo (+ ones1 x bo) -> fp32 -> DRAM.
"""

from contextlib import ExitStack

import ml_dtypes
import numpy as np

import concourse.bass as bass
import concourse.tile as tile
from concourse import bacc, mybir
from concourse.bass_utils import run_bass_kernel_spmd

F32 = mybir.dt.float32
BF16 = mybir.dt.bfloat16

HEADS = 8
DH = 64
N = 512
DIM = 512
SCALE = DH**-0.5
S = 4  # (b, p) slices per core
N_CORES = 8

SHUF0 = [0] * 32  # stream_shuffle mask: broadcast quadrant partition 0


def _build_body(ctx: ExitStack, tc: tile.TileContext, qT, kvT, wq, wk, wv, wo, bo, out):
    nc = tc.nc

    const = ctx.enter_context(tc.tile_pool(name="const", bufs=1))
    xT = ctx.enter_context(tc.tile_pool(name="xT", bufs=2))
    proj = ctx.enter_context(tc.tile_pool(name="proj", bufs=2))
    ptp = ctx.enter_context(tc.tile_pool(name="ptp", bufs=3))
    outTp = ctx.enter_context(tc.tile_pool(name="outTp", bufs=2))
    rbp = ctx.enter_context(tc.tile_pool(name="rbp", bufs=2))
    finp = ctx.enter_context(tc.tile_pool(name="finp", bufs=2))
    mm_ps = ctx.enter_context(tc.tile_pool(name="mm_ps", bufs=2, space="PSUM"))
    st_ps = ctx.enter_context(tc.tile_pool(name="st_ps", bufs=3, space="PSUM"))
    pv_ps = ctx.enter_context(tc.tile_pool(name="pv_ps", bufs=2, space="PSUM"))
    l_ps = ctx.enter_context(tc.tile_pool(name="l_ps", bufs=1, space="PSUM"))

    # --- weights (already bf16 in DRAM): [512, 512] -> [128, 4*512] ---
    w_sb = {}
    for name, dram in (("wq", wq), ("wk", wk), ("wv", wv), ("wo", wo)):
        w16 = const.tile([128, 4 * 512], BF16, name=f"{name}16")
        nc.sync.dma_start(w16[:], dram.rearrange("(t p) e -> p t e", p=128))
        w_sb[name] = w16
    bo16 = const.tile([1, 512], BF16, name="bo16")
    nc.sync.dma_start(bo16[:], bo.rearrange("(o f) -> o f", o=1))
    ones64 = const.tile([128, 64], BF16, name="ones64")
    nc.gpsimd.memset(ones64[:], 1.0)
    ones1 = const.tile([1, 128], BF16, name="ones1")
    nc.gpsimd.memset(ones1[:], 1.0)
    wq16, wk16, wv16, wo16 = (w_sb[k] for k in ("wq", "wk", "wv", "wo"))

    for s in range(S):
        # --- load pre-transposed bf16 inputs ---
        xqT = xT.tile([128, 4 * 512], BF16, name="xqT")
        nc.sync.dma_start(xqT[:], qT[s].rearrange("(t p) n -> p t n", p=128))
        xkvT = xT.tile([128, 4 * 512], BF16, name="xkvT")
        nc.sync.dma_start(xkvT[:], kvT[s].rearrange("(t p) n -> p t n", p=128))

        # --- projections ---
        qT16 = proj.tile([128, 4 * 512], BF16, name="qT16")
        kT16 = proj.tile([128, 4 * 512], BF16, name="kT16")
        v16 = proj.tile([128, 4 * 512], BF16, name="v16")
        for w16, xt, dst in ((wq16, xqT, qT16), (wk16, xkvT, kT16)):
            for t in range(4):  # output row-block (e)
                ps = mm_ps.tile([128, 512], F32, name="mm_ps")
                for d in range(4):  # contraction block
                    nc.tensor.matmul(
                        ps[:],
                        w16[:, d * 512 + t * 128 : d * 512 + (t + 1) * 128],
                        xt[:, d * 512 : (d + 1) * 512],
                        start=(d == 0),
                        stop=(d == 3),
                    )
                nc.vector.tensor_copy(dst[:, t * 512 : (t + 1) * 512], ps[:])
        for jb in range(4):  # v, normal layout: rows j, free e
            ps = mm_ps.tile([128, 512], F32, name="mm_ps")
            for d in range(4):
                nc.tensor.matmul(
                    ps[:],
                    xkvT[:, d * 512 + jb * 128 : d * 512 + (jb + 1) * 128],
                    wv16[:, d * 512 : (d + 1) * 512],
                    start=(d == 0),
                    stop=(d == 3),
                )
            nc.vector.tensor_copy(v16[:, jb * 512 : (jb + 1) * 512], ps[:])

        # --- attention, head pairs ---
        outT16 = outTp.tile([128, 4 * 512], BF16, name="outT16")
        for tp in range(4):
            h0, h1 = 2 * tp, 2 * tp + 1
            pts = []
            for h, half in ((h0, 0), (h1, 64)):
                kT_h = kT16[half : half + 64, tp * 512 : (tp + 1) * 512]
                qT_h = qT16[half : half + 64, tp * 512 : (tp + 1) * 512]
                pt16 = ptp.tile([128, 4 * 512], BF16, name="pt16")
                for jb in range(4):
                    stt = st_ps.tile([128, 512], F32, name="st_ps")
                    nc.tensor.matmul(
                        stt[:],
                        kT_h[:, jb * 128 : (jb + 1) * 128],
                        qT_h,
                        start=True,
                        stop=True,
                    )
                    nc.scalar.activation(
                        pt16[:, jb * 512 : (jb + 1) * 512],
                        stt[:],
                        mybir.ActivationFunctionType.Exp,
                        scale=SCALE,
                    )
                pts.append(pt16)
            pt_e, pt_o = pts

            # PV: column-group-paired issues so PE overlaps outT with l.
            pv = pv_ps.tile([128, 512], F32, name="pv_ps")
            lps = l_ps.tile([128, 512], F32, name="l_ps")
            for jb in range(4):
                pe_s = pt_e[:, jb * 512 : (jb + 1) * 512]
                po_s = pt_o[:, jb * 512 : (jb + 1) * 512]
                st, sp = (jb == 0), (jb == 3)
                # issue A: even outT (cols 0-63) || odd l-bcast (cols 64-127)
                nc.tensor.matmul(
                    pv[0:64, :],
                    v16[:, jb * 512 + h0 * 64 : jb * 512 + (h0 + 1) * 64],
                    pe_s, start=st, stop=sp, skip_group_check=True,
                )
                nc.tensor.matmul(
                    lps[64:128, :], ones64[:], po_s, start=st, stop=sp,
                    skip_group_check=True,
                )
                # issue B: odd outT (cols 64-127) || even l-bcast (cols 0-63)
                nc.tensor.matmul(
                    pv[64:128, :],
                    v16[:, jb * 512 + h1 * 64 : jb * 512 + (h1 + 1) * 64],
                    po_s, start=st, stop=sp, skip_group_check=True,
                )
                nc.tensor.matmul(
                    lps[0:64, :], ones64[:], pe_s, start=st, stop=sp,
                    skip_group_check=True,
                )
            for h, half in ((h0, 0), (h1, 64)):
                rb1 = rbp.tile([128, 512], F32, name="rb1")
                nc.vector.reciprocal(
                    rb1[half : half + 64, :], lps[half : half + 64, :]
                )
                nc.vector.tensor_mul(
                    outT16[half : half + 64, tp * 512 : (tp + 1) * 512],
                    pv[half : half + 64, :],
                    rb1[half : half + 64, :],
                )

        # --- final projection + bias ---
        fin = finp.tile([128, 4 * 512], F32, name="fin")
        for ib in range(4):
            ps = mm_ps.tile([128, 512], F32, name="mm_ps")
            for t in range(4):
                nc.tensor.matmul(
                    ps[:],
                    outT16[:, t * 512 + ib * 128 : t * 512 + (ib + 1) * 128],
                    wo16[:, t * 512 : (t + 1) * 512],
                    start=(t == 0),
                    stop=False,
                )
            nc.tensor.matmul(ps[:], ones1[:], bo16[:], start=False, stop=True)
            nc.scalar.copy(fin[:, ib * 512 : (ib + 1) * 512], ps[:])
        nc.sync.dma_start(out[s].rearrange("(a p) f -> p a f", p=128), fin[:])


def build_nc():
    nc = bacc.Bacc("TRN2", target_bir_lowering=False, debug=False)
    qT = nc.dram_tensor("qT", [S, DIM, N], BF16, kind="ExternalInput").ap()
    kvT = nc.dram_tensor("kvT", [S, DIM, N], BF16, kind="ExternalInput").ap()
    wq = nc.dram_tensor("wq", [DIM, DIM], BF16, kind="ExternalInput").ap()
    wk = nc.dram_tensor("wk", [DIM, DIM], BF16, kind="ExternalInput").ap()
    wv = nc.dram_tensor("wv", [DIM, DIM], BF16, kind="ExternalInput").ap()
    wo = nc.dram_tensor("wo", [DIM, DIM], BF16, kind="ExternalInput").ap()
    bo = nc.dram_tensor("bo", [DIM], BF16, kind="ExternalInput").ap()
    out = nc.dram_tensor("out", [S, N, DIM], F32, kind="ExternalOutput").ap()
    with tile.TileContext(nc) as tc:
        with ExitStack() as ctx:
            _build_body(ctx, tc, qT, kvT, wq, wk, wv, wo, bo, out)
    nc.compile()
    return nc


_NC = None
BF = ml_dtypes.bfloat16


def make_in_maps(q_in, kv_in, Wq, Wk, Wv, Wo, bo):
    # host-side layout prep: per-slice transpose to [dim, n] + bf16 cast
    q = np.asarray(q_in, dtype=np.float32).reshape(32, N, DIM)
    kv = np.asarray(kv_in, dtype=np.float32).reshape(32, N, DIM)
    qT = np.ascontiguousarray(q.transpose(0, 2, 1)).astype(BF)
    kvT = np.ascontiguousarray(kv.transpose(0, 2, 1)).astype(BF)
    w = {
        "wq": np.asarray(Wq, dtype=np.float32).astype(BF),
        "wk": np.asarray(Wk, dtype=np.float32).astype(BF),
        "wv": np.asarray(Wv, dtype=np.float32).astype(BF),
        "wo": np.asarray(Wo, dtype=np.float32).astype(BF),
        "bo": np.asarray(bo, dtype=np.float32).astype(BF),
    }
    return [
        {"qT": qT[S * c : S * (c + 1)], "kvT": kvT[S * c : S * (c + 1)], **w}
        for c in range(N_CORES)
    ]


def kernel(q_in, kv_in, Wq, Wk, Wv, Wo, bo):
    global _NC
    if _NC is None:
        _NC = build_nc()
    in_maps = make_in_maps(q_in, kv_in, Wq, Wk, Wv, Wo, bo)
    res = run_bass_kernel_spmd(_NC, in_maps, list(range(N_CORES))).results
    out = np.concatenate([res[c]["out"] for c in range(N_CORES)], axis=0)
    return out.reshape(4, 8, N, DIM)


# revision 10
# speedup vs baseline: 3.7743x; 3.7743x over previous
"""Trainium2 Bass kernel for CrossAttention (b=4, p=8, n=512, dim=512, 8 heads x 64).

Sharding: the 32 independent (b, p) slices are split 4-per-core across 8
NeuronCores (pure data parallel, no collectives). Weights are replicated.

Host-side prep (inside kernel()): inputs are transposed per-slice to
[dim, n] and cast to bf16, so the device never transposes activations;
weights are cast to bf16 on the host too.

Per-slice device dataflow (all SBUF tiles are [partition, free]):
  - qT = Wq-blocks^T @ xqT, kT likewise; v = xkvT-blocks^T @ Wv  (PE)
  - per head: ST[j, i] = kT_h-block^T @ qT_h -> exp (ACT, scale=1/8) -> PT bf16
    (no max subtraction: scores are ~N(0,1), exp cannot overflow)
  - PV for a head pair is issued column-group-paired so PE overlaps:
    even head outT -> PSUM[0:64] while odd head's l-matmul (ones[128,33])
    lands in PSUM[64:97] of a second bank, and vice versa. l rows at the
    two quadrant bases let a DVE stream_shuffle broadcast l across all 64
    partitions of the head's parity range, all partition-aligned.
  - normalize: outT_h *= 1/l_h (DVE reciprocal + stream_shuffle + mul)
  - final: fin[i, f] = outT-blocks^T @ Wo (+ ones1 x bo) -> fp32 -> DRAM.
"""

from contextlib import ExitStack

import ml_dtypes
import numpy as np

import concourse.bass as bass
import concourse.tile as tile
from concourse import bacc, mybir
from concourse.bass_utils import run_bass_kernel_spmd

F32 = mybir.dt.float32
BF16 = mybir.dt.bfloat16

HEADS = 8
DH = 64
N = 512
DIM = 512
SCALE = DH**-0.5
S = 4  # (b, p) slices per core
N_CORES = 8

SHUF0 = [0] * 32  # stream_shuffle mask: broadcast quadrant partition 0


def _build_body(ctx: ExitStack, tc: tile.TileContext, qT, kvT, wq, wk, wv, wo, bo, out):
    nc = tc.nc

    const = ctx.enter_context(tc.tile_pool(name="const", bufs=1))
    xT = ctx.enter_context(tc.tile_pool(name="xT", bufs=3))
    proj = ctx.enter_context(tc.tile_pool(name="proj", bufs=2))
    ptp = ctx.enter_context(tc.tile_pool(name="ptp", bufs=4))
    outTp = ctx.enter_context(tc.tile_pool(name="outTp", bufs=2))
    rbp = ctx.enter_context(tc.tile_pool(name="rbp", bufs=4))
    finp = ctx.enter_context(tc.tile_pool(name="finp", bufs=2))
    mm_ps = ctx.enter_context(tc.tile_pool(name="mm_ps", bufs=2, space="PSUM"))
    st_ps = ctx.enter_context(tc.tile_pool(name="st_ps", bufs=3, space="PSUM"))
    pv_ps = ctx.enter_context(tc.tile_pool(name="pv_ps", bufs=2, space="PSUM"))
    l_ps = ctx.enter_context(tc.tile_pool(name="l_ps", bufs=1, space="PSUM"))

    # --- weights (already bf16 in DRAM): [512, 512] -> [128, 4*512] ---
    w_sb = {}
    for name, dram in (("wq", wq), ("wk", wk), ("wv", wv), ("wo", wo)):
        w16 = const.tile([128, 4 * 512], BF16, name=f"{name}16")
        nc.sync.dma_start(w16[:], dram.rearrange("(t p) e -> p t e", p=128))
        w_sb[name] = w16
    bo16 = const.tile([1, 512], BF16, name="bo16")
    nc.sync.dma_start(bo16[:], bo.rearrange("(o f) -> o f", o=1))
    ones64 = const.tile([128, 64], BF16, name="ones64")
    nc.gpsimd.memset(ones64[:], 1.0)
    ones1 = const.tile([1, 128], BF16, name="ones1")
    nc.gpsimd.memset(ones1[:], 1.0)
    wq16, wk16, wv16, wo16 = (w_sb[k] for k in ("wq", "wk", "wv", "wo"))

    for s in range(S):
        # --- load pre-transposed bf16 inputs ---
        xqT = xT.tile([128, 4 * 512], BF16, name="xqT")
        nc.sync.dma_start(xqT[:], qT[s].rearrange("(t p) n -> p t n", p=128))
        xkvT = xT.tile([128, 4 * 512], BF16, name="xkvT")
        nc.sync.dma_start(xkvT[:], kvT[s].rearrange("(t p) n -> p t n", p=128))

        # --- projections ---
        qT16 = proj.tile([128, 4 * 512], BF16, name="qT16")
        kT16 = proj.tile([128, 4 * 512], BF16, name="kT16")
        v16 = proj.tile([128, 4 * 512], BF16, name="v16")
        for w16, xt, dst in ((wq16, xqT, qT16), (wk16, xkvT, kT16)):
            for t in range(4):  # output row-block (e)
                ps = mm_ps.tile([128, 512], F32, name="mm_ps")
                for d in range(4):  # contraction block
                    nc.tensor.matmul(
                        ps[:],
                        w16[:, d * 512 + t * 128 : d * 512 + (t + 1) * 128],
                        xt[:, d * 512 : (d + 1) * 512],
                        start=(d == 0),
                        stop=(d == 3),
                    )
                nc.vector.tensor_copy(dst[:, t * 512 : (t + 1) * 512], ps[:])
        for jb in range(4):  # v, normal layout: rows j, free e
            ps = mm_ps.tile([128, 512], F32, name="mm_ps")
            for d in range(4):
                nc.tensor.matmul(
                    ps[:],
                    xkvT[:, d * 512 + jb * 128 : d * 512 + (jb + 1) * 128],
                    wv16[:, d * 512 : (d + 1) * 512],
                    start=(d == 0),
                    stop=(d == 3),
                )
            nc.vector.tensor_copy(v16[:, jb * 512 : (jb + 1) * 512], ps[:])

        # --- attention, head pairs ---
        outT16 = outTp.tile([128, 4 * 512], BF16, name="outT16")
        for tp in range(4):
            h0, h1 = 2 * tp, 2 * tp + 1
            pts = []
            for h, half in ((h0, 0), (h1, 64)):
                kT_h = kT16[half : half + 64, tp * 512 : (tp + 1) * 512]
                qT_h = qT16[half : half + 64, tp * 512 : (tp + 1) * 512]
                pt16 = ptp.tile([128, 4 * 512], BF16, name="pt16")
                for jb in range(4):
                    stt = st_ps.tile([128, 512], F32, name="st_ps")
                    nc.tensor.matmul(
                        stt[:],
                        kT_h[:, jb * 128 : (jb + 1) * 128],
                        qT_h,
                        start=True,
                        stop=True,
                    )
                    nc.scalar.activation(
                        pt16[:, jb * 512 : (jb + 1) * 512],
                        stt[:],
                        mybir.ActivationFunctionType.Exp,
                        scale=SCALE,
                    )
                pts.append(pt16)
            pt_e, pt_o = pts

            # PV: column-group-paired issues so PE overlaps outT with l.
            pv = pv_ps.tile([128, 512], F32, name="pv_ps")
            lps = l_ps.tile([128, 512], F32, name="l_ps")
            for jb in range(4):
                pe_s = pt_e[:, jb * 512 : (jb + 1) * 512]
                po_s = pt_o[:, jb * 512 : (jb + 1) * 512]
                st, sp = (jb == 0), (jb == 3)
                # issue A: even outT (cols 0-63) || odd l-bcast (cols 64-127)
                nc.tensor.matmul(
                    pv[0:64, :],
                    v16[:, jb * 512 + h0 * 64 : jb * 512 + (h0 + 1) * 64],
                    pe_s, start=st, stop=sp, skip_group_check=True,
                )
                nc.tensor.matmul(
                    lps[64:128, :], ones64[:], po_s, start=st, stop=sp,
                    skip_group_check=True,
                )
                # issue B: odd outT (cols 64-127) || even l-bcast (cols 0-63)
                nc.tensor.matmul(
                    pv[64:128, :],
                    v16[:, jb * 512 + h1 * 64 : jb * 512 + (h1 + 1) * 64],
                    po_s, start=st, stop=sp, skip_group_check=True,
                )
                nc.tensor.matmul(
                    lps[0:64, :], ones64[:], pe_s, start=st, stop=sp,
                    skip_group_check=True,
                )
            for h, half in ((h0, 0), (h1, 64)):
                rb1 = rbp.tile([128, 512], F32, name="rb1")
                nc.vector.reciprocal(
                    rb1[half : half + 64, :], lps[half : half + 64, :]
                )
                nc.vector.tensor_mul(
                    outT16[half : half + 64, tp * 512 : (tp + 1) * 512],
                    pv[half : half + 64, :],
                    rb1[half : half + 64, :],
                )

        # --- final projection + bias ---
        fin = finp.tile([128, 4 * 512], F32, name="fin")
        for ib in range(4):
            ps = mm_ps.tile([128, 512], F32, name="mm_ps")
            for t in range(4):
                nc.tensor.matmul(
                    ps[:],
                    outT16[:, t * 512 + ib * 128 : t * 512 + (ib + 1) * 128],
                    wo16[:, t * 512 : (t + 1) * 512],
                    start=(t == 0),
                    stop=False,
                )
            nc.tensor.matmul(ps[:], ones1[:], bo16[:], start=False, stop=True)
            nc.vector.tensor_copy(fin[:, ib * 512 : (ib + 1) * 512], ps[:])
        nc.sync.dma_start(out[s].rearrange("(a p) f -> p a f", p=128), fin[:])


def build_nc():
    nc = bacc.Bacc("TRN2", target_bir_lowering=False, debug=False)
    qT = nc.dram_tensor("qT", [S, DIM, N], BF16, kind="ExternalInput").ap()
    kvT = nc.dram_tensor("kvT", [S, DIM, N], BF16, kind="ExternalInput").ap()
    wq = nc.dram_tensor("wq", [DIM, DIM], BF16, kind="ExternalInput").ap()
    wk = nc.dram_tensor("wk", [DIM, DIM], BF16, kind="ExternalInput").ap()
    wv = nc.dram_tensor("wv", [DIM, DIM], BF16, kind="ExternalInput").ap()
    wo = nc.dram_tensor("wo", [DIM, DIM], BF16, kind="ExternalInput").ap()
    bo = nc.dram_tensor("bo", [DIM], BF16, kind="ExternalInput").ap()
    out = nc.dram_tensor("out", [S, N, DIM], F32, kind="ExternalOutput").ap()
    with tile.TileContext(nc) as tc:
        with ExitStack() as ctx:
            _build_body(ctx, tc, qT, kvT, wq, wk, wv, wo, bo, out)
    nc.compile()
    return nc


_NC = None
BF = ml_dtypes.bfloat16


def make_in_maps(q_in, kv_in, Wq, Wk, Wv, Wo, bo):
    # host-side layout prep: per-slice transpose to [dim, n] + bf16 cast
    q = np.asarray(q_in, dtype=np.float32).reshape(32, N, DIM)
    kv = np.asarray(kv_in, dtype=np.float32).reshape(32, N, DIM)
    qT = np.ascontiguousarray(q.transpose(0, 2, 1)).astype(BF)
    kvT = np.ascontiguousarray(kv.transpose(0, 2, 1)).astype(BF)
    w = {
        "wq": np.asarray(Wq, dtype=np.float32).astype(BF),
        "wk": np.asarray(Wk, dtype=np.float32).astype(BF),
        "wv": np.asarray(Wv, dtype=np.float32).astype(BF),
        "wo": np.asarray(Wo, dtype=np.float32).astype(BF),
        "bo": np.asarray(bo, dtype=np.float32).astype(BF),
    }
    return [
        {"qT": qT[S * c : S * (c + 1)], "kvT": kvT[S * c : S * (c + 1)], **w}
        for c in range(N_CORES)
    ]


def kernel(q_in, kv_in, Wq, Wk, Wv, Wo, bo):
    global _NC
    if _NC is None:
        _NC = build_nc()
    in_maps = make_in_maps(q_in, kv_in, Wq, Wk, Wv, Wo, bo)
    res = run_bass_kernel_spmd(_NC, in_maps, list(range(N_CORES))).results
    out = np.concatenate([res[c]["out"] for c in range(N_CORES)], axis=0)
    return out.reshape(4, 8, N, DIM)
